# revision 40
# baseline (speedup 1.0000x reference)
"""AttentionBlstmQuora on 8 trn2 cores: data-parallel over batch (8 seq/core).

Transposed on-chip layout (feature dim on SBUF partitions, batch on free dim).
LSTM step restructured for latency: gate pre-activations land directly in PSUM
(xp preloaded via identity-matmul, Wh matmuls accumulate on top), gates grouped
[i, 2g | f, o] across two PSUM banks so one sigmoid covers i and 2g (tanh(g) =
2*sigmoid(2g)-1, the 2x baked into the weights host-side) and fires after only
half the burst. Backward-direction facts are stored time-reversed so both
directions' h-writes fuse into one DVE op; consumers read the bwd chunks
through negative-stride views.
"""

import numpy as np
import ml_dtypes

import concourse.bass as bass
import concourse.bacc as bacc
import concourse.mybir as mybir
import concourse.tile as tile
from concourse import bass_utils
from concourse.masks import make_identity

B, T, V, E, H, D, NH = 64, 121, 100000, 300, 256, 512, 3
NC = 8
BL = B // NC            # 8 sequences per core
BT = BL * T             # 968
G4 = 4 * H              # 1024
NHALF = BT // 2         # 484
TC0 = 60                # phase-B time split (unblocks LSTM steps 0..TC0-1)
EK = [128, 128, E - 256]
F32 = mybir.dt.float32
BF16 = mybir.dt.bfloat16
I32 = mybir.dt.int32
AF = mybir.ActivationFunctionType
OP = mybir.AluOpType

_CACHE = {}


def _build():
    nc = bacc.Bacc("TRN2", target_bir_lowering=False, debug=False, num_devices=NC)

    def dt(name, shape, dtype, kind="ExternalInput"):
        return nc.dram_tensor(name, shape, dtype, kind=kind).ap()

    d_tok = dt("tokT", [T, BL], I32)
    d_emb = dt("emb", [V, E], F32)
    d_mask = dt("negmask", [T, BL], F32)
    d_q = dt("qT", [128, 4 * BL], F32)
    d_wx = dt("wx", [2, E + 1, G4], BF16)
    d_wh = dt("wh", [2, H, G4], BF16)
    d_w1 = dt("w1", [24 * 128, E], BF16)  # 16 original chunks + 8 hop0-combined
    d_b1 = dt("b1T", [128, 3], F32)
    d_w2 = dt("w2", [128, 3], BF16)
    d_wrc = dt("wrc", [2, D + 1, D], BF16)
    d_uu = dt("uu", [2, D, D], BF16)
    d_whop = dt("whops", [NH, 12 * 128, D], BF16)
    d_bhop = dt("bhopT", [128, NH * 4], F32)
    d_wo = dt("wo", [128, 8], BF16)
    d_sel = dt("sel", [BL, BL * 128], BF16)
    d_bo = dt("bo", [1, 1], F32)
    d_out = dt("out", [1, BL], F32, kind="ExternalOutput")

    with tile.TileContext(nc) as tc:
        cp = tc.alloc_tile_pool(name="const", bufs=1)
        wp = tc.alloc_tile_pool(name="work", bufs=2)
        pp = tc.alloc_tile_pool(name="ps", bufs=1, space="PSUM")
        pp2 = tc.alloc_tile_pool(name="ps2", bufs=2, space="PSUM")

        ident = cp.tile([128, 128], F32, name="ident")
        make_identity(nc, ident[:])
        identb = cp.tile([128, 128], BF16, name="identb")
        nc.vector.tensor_copy(identb[:], ident[:])

        tok_sb = cp.tile([T, BL], I32, name="tok")
        nc.sync.dma_start(tok_sb[:], d_tok)
        mask_sb = cp.tile([T, BL], F32, name="mask")
        nc.sync.dma_start(mask_sb[:], d_mask)
        q_sb = cp.tile([128, 4 * BL], F32, name="q")
        nc.sync.dma_start(q_sb[:], d_q)
        q_bf = cp.tile([128, 4 * BL], BF16, name="qbf")
        nc.vector.tensor_copy(q_bf[:], q_sb[:])

        wx_sb = [cp.tile([EK[k] + (1 if k == 2 else 0), 2 * G4], BF16, name=f"wx{k}")
                 for k in range(3)]
        for k in range(3):
            rows = EK[k] + (1 if k == 2 else 0)
            for d_ in range(2):
                nc.sync.dma_start(wx_sb[k][:, d_ * G4:(d_ + 1) * G4],
                                  d_wx[d_, k * 128:k * 128 + rows, :])
        wh_sb = [cp.tile([128, 2 * G4], BF16, name=f"wh{k}") for k in range(2)]
        for k in range(2):
            for d_ in range(2):
                nc.sync.dma_start(wh_sb[k][:, d_ * G4:(d_ + 1) * G4],
                                  d_wh[d_, k * 128:(k + 1) * 128, :])
        w1_sb = cp.tile([128, 24 * E], BF16, name="w1")
        for k in range(24):
            nc.sync.dma_start(w1_sb[:, k * E:(k + 1) * E], d_w1[k * 128:(k + 1) * 128, :])
        b1_sb = cp.tile([128, 3], F32, name="b1")
        nc.sync.dma_start(b1_sb[:], d_b1)
        w2_sb = cp.tile([128, 3], BF16, name="w2")
        nc.sync.dma_start(w2_sb[:], d_w2)
        wrc_sb = cp.tile([128, 2 * 4 * D], BF16, name="wrc")
        wrcb_sb = cp.tile([1, 2 * D], BF16, name="wrcb")
        for rc in range(2):
            for k in range(4):
                nc.sync.dma_start(wrc_sb[:, (rc * 4 + k) * D:(rc * 4 + k + 1) * D],
                                  d_wrc[rc, k * 128:(k + 1) * 128, :])
            nc.sync.dma_start(wrcb_sb[:, rc * D:(rc + 1) * D], d_wrc[rc, D:D + 1, :])
        uu_sb = cp.tile([128, 2 * 4 * D], BF16, name="uu")
        for rc in range(2):
            for k in range(4):
                nc.sync.dma_start(uu_sb[:, (rc * 4 + k) * D:(rc * 4 + k + 1) * D],
                                  d_uu[rc, k * 128:(k + 1) * 128, :])
        bhop_sb = cp.tile([128, NH * 4], F32, name="bhop")
        nc.sync.dma_start(bhop_sb[:], d_bhop)
        wo_sb = cp.tile([128, 8], BF16, name="wo")
        nc.sync.dma_start(wo_sb[:], d_wo)
        bo_sb = cp.tile([1, 1], F32, name="bo")
        nc.sync.dma_start(bo_sb[:], d_bo)
        sel_sb = cp.tile([BL, BL * 128], BF16, name="sel")
        nc.sync.dma_start(sel_sb[:], d_sel)
        onesrow = cp.tile([1, NHALF], BF16, name="onesrow")
        nc.gpsimd.memset(onesrow[:], 1.0)

        # ---- phase A: gather + transpose x ----
        xT = [cp.tile([EK[k] + (1 if k == 2 else 0), BT], BF16, name=f"xT{k}")
              for k in range(3)]
        nc.gpsimd.memset(xT[2][:], 1.0)  # row 44 stays 1.0 (bias row)
        with tc.tile_pool(name="gather", bufs=2) as gp:
            for b in range(BL):
                xg = gp.tile([T, E], F32, tag="xg")
                nc.gpsimd.indirect_dma_start(
                    out=xg[:], out_offset=None, in_=d_emb,
                    in_offset=bass.IndirectOffsetOnAxis(ap=tok_sb[:, b:b + 1], axis=0),
                )
                for k in range(3):
                    pt = pp2.tile([EK[k], T], F32, tag="mm", space="PSUM")
                    nc.tensor.transpose(pt[:], xg[:, k * 128:k * 128 + EK[k]],
                                        ident[:T, :T])
                    nc.scalar.activation(xT[k][0:EK[k], b * T:(b + 1) * T], pt[:],
                                         AF.Copy)

        # ---- phase B: xp = x @ Wx + b, layout (d c b t); bwd (d=1) time-reversed
        xp = cp.tile([128, 16 * BT], BF16, name="xp")
        xpv = xp.rearrange("p (d c b t) -> p d c b t", d=2, c=8, b=BL)
        xTv = [xT[k].rearrange("r (b t) -> r b t", b=BL) for k in range(3)]
        # wave 0 produces STORED cols [0, TC0) for both dirs (bwd stores
        # reversed, so its wave 0 consumes t in [T-TC0, T)), unblocking LSTM
        # steps 0..TC0-1.  Wave 1 is emitted INSIDE the LSTM loop (one group
        # per step) so its psum->SBUF copies fill Scalar slack instead of
        # queueing ahead of the first steps' activations.
        def emit_xp_group(d_, c, t0, t1):
            ps = pp2.tile([128, BL * (t1 - t0)], F32, tag="mm", space="PSUM",
                          name="xpg")
            psv = ps.rearrange("p (b t) -> p b t", b=BL)
            for k in range(3):
                rows = EK[k] + (1 if k == 2 else 0)
                nc.tensor.matmul(
                    psv[:],
                    wx_sb[k][:rows, d_ * G4 + c * 128:d_ * G4 + (c + 1) * 128],
                    xTv[k][:rows, :, t0:t1],
                    start=(k == 0), stop=(k == 2))
            if d_ == 0:
                out = xpv[:, 0, c, :, t0:t1]
            else:
                out = xpv[:, 1, c, :, ::-1][:, :, t0:t1]
            nc.scalar.activation(out, psv[:], AF.Copy)

        for d_ in range(2):
            t0, t1 = (0, TC0) if d_ == 0 else (T - TC0, T)
            for c in range(8):
                emit_xp_group(d_, c, t0, t1)
        wave1 = [(d_, c, (TC0, T) if d_ == 0 else (0, T - TC0))
                 for d_ in range(2) for c in range(8)]

        # ---- phase C: BiLSTM; facts col = dk*BT + b*T + s (bwd chunks dk=2,3
        #      stored time-REVERSED: col s holds h_bwd(T-1-s))
        facts = cp.tile([128, 4 * BT], BF16, name="facts")
        factsv = facts.rearrange("p (dk b t) -> p dk b t", dk=4, b=BL)
        factsv2 = facts.rearrange("p (d k b t) -> p d k b t", d=2, k=2, b=BL)
        h0 = cp.tile([128, 4 * BL], BF16, name="h0")
        nc.gpsimd.memset(h0[:], 0.0)
        h0v = h0.rearrange("p (dk b) -> p dk b", dk=4)
        # recurrence reads h from a small contiguous tile; the strided facts
        # write happens off the critical path on GpSimd
        hcur = [cp.tile([128, 4 * BL], BF16, name=f"hcur{i}") for i in range(2)]
        hcurv = [hh.rearrange("p (d k b) -> p d k b", d=2, k=2) for hh in hcur]
        cst = cp.tile([128, 32], F32, name="cst")
        nc.gpsimd.memset(cst[:], 0.0)
        cstv = cst.rearrange("p (d k b) -> p d k b", d=2, k=2)

        psA = pp.tile([128, 512], F32, name="psA", space="PSUM")
        psB = pp.tile([128, 512], F32, name="psB", space="PSUM")
        psAv = psA[:, 0:64].rearrange("p (d c b) -> p d c b", d=2, c=4)
        psBv = psB[:, 0:64].rearrange("p (d c b) -> p d c b", d=2, c=4)
        sA = cp.tile([128, 64], F32, name="sA")
        sB = cp.tile([128, 64], F32, name="sB")
        sAv = sA.rearrange("p (d c b) -> p d c b", d=2, c=2 * 2)
        sBv = sB.rearrange("p (d c b) -> p d c b", d=2, c=2 * 2)

        for s in range(T):
            # tensor: preload xp (identity matmul) then accumulate Wh.T @ h
            for bank, psv, clo in ((0, psAv, 0), (1, psBv, 4)):
                nc.tensor.matmul(psv[:, :, :, :], identb[:],
                                 xpv[:, :, clo:clo + 4, :, s],
                                 start=True, stop=False, skip_group_check=True)
                for d_ in range(2):
                    for ci in range(4):
                        c = clo + ci
                        for k in range(2):
                            dk = d_ * 2 + k
                            rhs = (h0v[:, dk, :] if s == 0
                                   else hcurv[(s - 1) % 2][:, d_, k, :])
                            nc.tensor.matmul(
                                psv[:, d_, ci, :],
                                wh_sb[k][:, d_ * G4 + c * 128:d_ * G4 + (c + 1) * 128],
                                rhs, start=False,
                                stop=(d_ == 1 and ci == 3 and k == 1),
                                skip_group_check=True)
            nc.scalar.activation(sA[:], psA[:, 0:64], AF.Sigmoid)
            nc.scalar.activation(sB[:], psB[:, 0:64], AF.Sigmoid)
            # m = sig_i * sig_2g ; u = 2m - sig_i ; c' = sig_f*c + u ; h = sig_o*tanh(c')
            mm_ = wp.tile([128, 32], F32, tag="mm_")
            mmv = mm_.rearrange("p (d k b) -> p d k b", d=2, k=2)
            nc.vector.tensor_tensor(mmv[:], sAv[:, :, 0:2, :], sAv[:, :, 2:4, :],
                                    op=OP.mult)
            ut = wp.tile([128, 32], F32, tag="ut")
            utv = ut.rearrange("p (d k b) -> p d k b", d=2, k=2)
            nc.vector.scalar_tensor_tensor(utv[:], mmv[:], 2.0, sAv[:, :, 0:2, :],
                                           op0=OP.mult, op1=OP.subtract)
            vt = wp.tile([128, 32], F32, tag="vt")
            vtv = vt.rearrange("p (d k b) -> p d k b", d=2, k=2)
            nc.vector.tensor_tensor(vtv[:], sBv[:, :, 0:2, :], cstv[:], op=OP.mult)
            nc.vector.tensor_tensor(cst[:], vt[:], ut[:], op=OP.add)
            th = wp.tile([128, 32], F32, tag="th")
            nc.scalar.activation(th[:], cst[:], AF.Tanh)
            thv = th.rearrange("p (d k b) -> p d k b", d=2, k=2)
            nc.vector.tensor_tensor(hcurv[s % 2][:], sBv[:, :, 2:4, :],
                                    thv[:], op=OP.mult)
            nc.gpsimd.tensor_copy(factsv2[:, :, :, :, s], hcurv[s % 2][:])
            if s >= 2 and s % 2 == 0 and (s - 2) // 2 < len(wave1):
                dw, cw, (tw0, tw1) = wave1[(s - 2) // 2]
                emit_xp_group(dw, cw, tw0, tw1)

        # ---- GRU precompute: xr/xc = facts @ W(r|c) + b (transposed) ----
        # bwd chunks (k=2,3) of facts are stored reversed -> reversed rhs views
        xrc = cp.tile([128, 2 * 4 * BT], BF16, name="xrc")
        for rc in range(2):
            for c in range(4):
                for h_ in range(2):
                    ps = pp2.tile([128, NHALF], F32, tag="mm", space="PSUM")
                    for k in range(4):
                        if k < 2:
                            rhs = facts[:, k * BT + h_ * NHALF:k * BT + (h_ + 1) * NHALF]
                        else:
                            rhs = factsv[:, k, h_ * 4:(h_ + 1) * 4, ::-1]
                        nc.tensor.matmul(
                            ps[:], wrc_sb[:, (rc * 4 + k) * D + c * 128:
                                          (rc * 4 + k) * D + (c + 1) * 128],
                            rhs, start=(k == 0), stop=False)
                    nc.tensor.matmul(
                        ps[:], wrcb_sb[0:1, rc * D + c * 128:rc * D + (c + 1) * 128],
                        onesrow[0:1, :], start=False, stop=True)
                    nc.scalar.activation(
                        xrc[:, (rc * 4 + c) * BT + h_ * NHALF:
                            (rc * 4 + c) * BT + (h_ + 1) * NHALF],
                        ps[:], AF.Copy)

        # ---- z pieces: zq/zaq constant across hops (stored in facts layout,
        #      so bwd chunks inherit the reversed time order)
        frr = factsv

        def make_z(zmul, zabs, mtile):
            # zm chunks first (they feed the first W1 kt's), then per-k
            # sub+abs so zam chunk k unblocks its consumers incrementally
            zm_r = zmul.rearrange("p (k b t) -> p k b t", k=4, b=BL)
            za_r = zabs.rearrange("p (k b t) -> p k b t", k=4, b=BL)
            m_r = mtile.rearrange("p (k b) -> p k b", k=4)
            for k in range(4):
                mb = m_r[:, k, :].to_broadcast([128, BL, T])
                nc.vector.tensor_tensor(zm_r[:, k, :, :], frr[:, k, :, :], mb,
                                        op=OP.mult)
            for k in range(4):
                mb = m_r[:, k, :].to_broadcast([128, BL, T])
                nc.vector.tensor_tensor(za_r[:, k, :, :], frr[:, k, :, :], mb,
                                        op=OP.subtract)
                # abs on the (otherwise idle) scalar engine, off the DVE chain
                nc.scalar.activation(za_r[:, k, :, :], za_r[:, k, :, :], AF.Abs)

        zq = cp.tile([128, 4 * BT], BF16, name="zq")
        zaq = cp.tile([128, 4 * BT], BF16, name="zaq")
        make_z(zq, zaq, q_bf)
        zm = cp.tile([128, 4 * BT], BF16, name="zm")
        zam = cp.tile([128, 4 * BT], BF16, name="zam")
        m_cur = cp.tile([128, 4 * BL], BF16, name="mcur")
        nc.vector.tensor_copy(m_cur[:], q_bf[:])

        whop_sb = cp.tile([128, 12 * D], BF16, name="whop")
        hgbuf = [cp.tile([128, 4 * BL], BF16, name=f"hgb{i}") for i in range(2)]
        dltg = [cp.tile([128, 4 * BL], BF16, name=f"dltg{i}") for i in range(2)]
        G4b = cp.tile([128, 4 * BT], BF16, name="G4b")   # col = b*T*4 + t*4 + k
        G4bv = G4b.rearrange("p (b t k) -> p b t k", b=BL, t=T)
        G4bk = G4b.rearrange("p (b t k) -> p k b t", b=BL, t=T)
        hatt = [cp.tile([EK[k], BT], BF16, name=f"hatt{k}") for k in range(3)]
        xrr = xrc.rearrange("p (rc c b t) -> p rc c b t", rc=2, c=4, b=BL)
        # GRU psum shares banks with the (temporally disjoint) LSTM psum:
        # psR in psA's bank, psC in psB's bank, distinct column regions.
        psR = psA[:, 256:288]
        psC = psB[:, 256:288]
        psRv = psR.rearrange("p (c b) -> p c b", c=4)
        psCv = psC.rearrange("p (c b) -> p c b", c=4)

        def zchunk(zt_, sub, h_):
            # rhs view for W1 contraction chunk `sub` of z piece zt_, half h_
            if sub < 2:
                return zt_[:, sub * BT + h_ * NHALF:sub * BT + (h_ + 1) * NHALF]
            return zt_.rearrange("p (k b t) -> p k b t", k=4, b=BL)[
                :, sub, h_ * 4:(h_ + 1) * 4, ::-1]

        part_sb = [cp.tile([EK[k], BT], BF16, name=f"part{k}") for k in range(3)]

        for hop in range(NH):
            nc.sync.dma_start(whop_sb[:].rearrange("p (k d) -> p k d", k=12),
                              d_whop[hop].rearrange("(k p) d -> p k d", p=128))
            if hop > 0:
                make_z(zm, zam, m_cur)
            # h_att^T = tanh(W1.T @ z^T + b1).  hop 0: combined (W1a+W1b |
            # W1c+W1d) chunks (kt 16..23) over [zq, zaq] only.  hops 1,2:
            # identity-preload the q-half partial (computed during hop 0's
            # GRU) + the 8 zm/zam chunks.
            for mc in range(3):
                rows = EK[mc]
                for h_ in range(2):
                    ps = pp2.tile([128, NHALF], F32, tag="mm", space="PSUM")
                    if hop == 0:
                        for j in range(8):
                            kt = 16 + j
                            blk, sub = j // 4, j % 4
                            nc.tensor.matmul(
                                ps[:rows, :],
                                w1_sb[:, kt * E + mc * 128:kt * E + mc * 128 + rows],
                                zchunk(zq if blk == 0 else zaq, sub, h_),
                                start=(j == 0), stop=(j == 7))
                    else:
                        nc.tensor.matmul(
                            ps[:rows, :], identb[:rows, :rows],
                            part_sb[mc][:rows, h_ * NHALF:(h_ + 1) * NHALF],
                            start=True, stop=False, skip_group_check=True)
                        for j, kt in enumerate((4, 5, 6, 7, 12, 13, 14, 15)):
                            blk, sub = kt // 4, kt % 4
                            nc.tensor.matmul(
                                ps[:rows, :],
                                w1_sb[:, kt * E + mc * 128:kt * E + mc * 128 + rows],
                                zchunk(zm if blk == 1 else zam, sub, h_),
                                start=False, stop=(j == 7), skip_group_check=True)
                    nc.scalar.activation(hatt[mc][:, h_ * NHALF:(h_ + 1) * NHALF],
                                         ps[:rows, :], AF.Tanh,
                                         bias=b1_sb[0:rows, mc:mc + 1])
            # s^T [T, BL] -> masked softmax in [BL, T]
            ps_s = pp2.tile([T, BL], F32, tag="small", space="PSUM")
            for b in range(BL):
                for k in range(3):
                    nc.tensor.matmul(ps_s[:, b:b + 1], hatt[k][:, b * T:(b + 1) * T],
                                     w2_sb[0:EK[k], k:k + 1],
                                     start=(k == 0), stop=(k == 2))
            # exp-free softmax, computed in [T, BL] so the elementwise
            # reciprocal runs across 121 partitions (8 elems/lane, not 121):
            # e^s = v/(1-v) with v = sigmoid(s) -- avoids the exp<->sigmoid
            # ACT table swap (~2.6us per hop).  |s| is bounded (tanh'd
            # features x small W2); masked positions give v=0 -> e=0 exactly.
            spreT = wp.tile([T, BL], F32, tag="spreT")
            nc.vector.tensor_tensor(spreT[:], ps_s[:], mask_sb[:], op=OP.add)
            vT = wp.tile([T, BL], F32, tag="vT")
            nc.scalar.activation(vT[:], spreT[:], AF.Sigmoid)
            wT = wp.tile([T, BL], F32, tag="wT")
            nc.vector.tensor_scalar(wT[:], vT[:], -1.0, 1.0,
                                    op0=OP.mult, op1=OP.add)
            rwT = wp.tile([T, BL], F32, tag="rwT")
            nc.vector.reciprocal(rwT[:], wT[:])
            eT = wp.tile([T, BL], F32, tag="eT")
            nc.vector.tensor_tensor(eT[:], vT[:], rwT[:], op=OP.mult)
            ps_e = pp2.tile([BL, T], F32, tag="small", space="PSUM")
            nc.tensor.transpose(ps_e[:], eT[:], ident[:T, :T])
            zsum = wp.tile([BL, 1], F32, tag="zsum")
            nc.vector.tensor_reduce(zsum[:], ps_e[:], axis=mybir.AxisListType.X,
                                    op=OP.add)
            rz = wp.tile([BL, 1], F32, tag="rz")
            nc.vector.reciprocal(rz[:], zsum[:])
            a_sb = wp.tile([BL, T], BF16, tag="asb")
            nc.vector.tensor_scalar_mul(a_sb[:], ps_e[:], rz[:])
            # normalized attention weights, k-replicated on all 128 partitions;
            # (b t k) layout so the fanout ACT writes contiguously
            for half in range(2):
                ps_gh = pp2.tile([128, 4 * T], F32, tag="mm", space="PSUM")
                for bi in range(4):
                    b = half * 4 + bi
                    nc.tensor.matmul(ps_gh[:, bi * T:(bi + 1) * T],
                                     sel_sb[:, b * 128:(b + 1) * 128], a_sb[:],
                                     start=True, stop=True, skip_group_check=True)
                src = ps_gh.rearrange("p (b t) -> p b t", b=4).to_broadcast(
                    [128, 4, T, 4])
                nc.scalar.activation(G4bv[:, half * 4:(half + 1) * 4, :, :], src,
                                     AF.Copy)
            # GRU over t: psum(t) = xr_t + Uu@hg_{t-1}, built as identity-preload
            # + Uu@hg_{t-2} (early, off critical path) + Uu@delta_{t-1} (after
            # the previous step's gating); hg update itself is off-path.
            nc.gpsimd.memset(hgbuf[0][:], 0.0)
            nc.gpsimd.memset(hgbuf[1][:], 0.0)
            for t in range(T):
                par, parp = t % 2, (t + 1) % 2
                hgA = hgbuf[par]       # hg_{t-2}; becomes hg_t at step end
                hgP = hgbuf[parp]      # hg_{t-1}
                dgp = dltg[parp].rearrange("p (k b) -> p k b", k=4)  # delta_{t-1}
                hgAv = hgA.rearrange("p (k b) -> p k b", k=4)
                if t > 0:
                    nc.tensor.matmul(psRv[:], identb[:], xrr[:, 0, :, :, t],
                                     start=True, stop=False, skip_group_check=True)
                    if t >= 2:
                        for half, psv in ((0, psRv), (1, psCv)):
                            for c in range(4):
                                for k in range(4):
                                    nc.tensor.matmul(
                                        psv[:, c, :],
                                        uu_sb[:, (half * 4 + k) * D + c * 128:
                                              (half * 4 + k) * D + (c + 1) * 128],
                                        hgAv[:, k, :],
                                        start=(half == 1 and c == 0 and k == 0),
                                        stop=False, skip_group_check=True)
                    for half, psv in ((0, psRv), (1, psCv)):
                        for c in range(4):
                            for k in range(4):
                                nc.tensor.matmul(
                                    psv[:, c, :],
                                    uu_sb[:, (half * 4 + k) * D + c * 128:
                                          (half * 4 + k) * D + (c + 1) * 128],
                                    dgp[:, k, :],
                                    start=(half == 1 and t == 1 and c == 0 and k == 0),
                                    stop=(c == 3 and k == 3),
                                    skip_group_check=True)
                # hop 0: compute the hop-invariant q-half W1 partials in the
                # tensor slack at the tail of each step's MM block
                if hop == 0 and 30 <= t < 54:
                    gi, sg = (t - 30) // 4, (t - 30) % 4
                    mc_, hh = gi // 2, gi % 2
                    rows_ = EK[mc_]
                    if sg == 0:
                        part_ps = pp2.tile([128, NHALF], F32, tag="mm",
                                           space="PSUM")
                    for jj in (2 * sg, 2 * sg + 1):
                        kt = (0, 1, 2, 3, 8, 9, 10, 11)[jj]
                        blk, sub = kt // 4, kt % 4
                        nc.tensor.matmul(
                            part_ps[:rows_, :],
                            w1_sb[:, kt * E + mc_ * 128:kt * E + mc_ * 128 + rows_],
                            zchunk(zq if blk == 0 else zaq, sub, hh),
                            start=(jj == 0), stop=(jj == 7),
                            skip_group_check=True)
                    if sg == 3:
                        nc.scalar.activation(
                            part_sb[mc_][:rows_, hh * NHALF:(hh + 1) * NHALF],
                            part_ps[:rows_, :], AF.Copy)
                if t == 0:
                    hc = wp.tile([128, 32], BF16, tag="hc")
                    nc.scalar.activation(hc.rearrange("p (c b) -> p c b", c=4),
                                         xrr[:, 1, :, :, 0], AF.Tanh)
                    dl = hc
                else:
                    rr = wp.tile([128, 32], F32, tag="rr")
                    nc.scalar.activation(rr[:], psR, AF.Sigmoid)
                    hcp = wp.tile([128, 32], F32, tag="hcp")
                    nc.vector.tensor_tensor(hcp[:], rr[:], psC, op=OP.mult)
                    hcp2 = wp.tile([128, 32], F32, tag="hcp2")
                    nc.vector.tensor_tensor(hcp2.rearrange("p (c b) -> p c b", c=4),
                                            hcp.rearrange("p (c b) -> p c b", c=4),
                                            xrr[:, 1, :, :, t], op=OP.add)
                    hc = wp.tile([128, 32], BF16, tag="hc")
                    nc.scalar.activation(hc[:], hcp2[:], AF.Tanh)
                    dl = wp.tile([128, 32], BF16, tag="dl")
                    nc.vector.tensor_tensor(dl[:], hc[:], hgP[:], op=OP.subtract)
                nc.vector.tensor_tensor(
                    dltg[par].rearrange("p (k b) -> p k b", k=4),
                    dl.rearrange("p (k b) -> p k b", k=4),
                    G4bk[:, :, :, t], op=OP.mult)
                nc.vector.tensor_tensor(hgA[:], hgP[:], dltg[par][:], op=OP.add)
            # m' = relu(Whop.T @ [m; ep; q] + bhop)
            ps_m = pp.tile([128, 32], F32, tag="lb", space="PSUM")
            rhs_t = [m_cur, hgbuf[0], q_bf]
            for mc in range(4):
                for kt in range(12):
                    src = rhs_t[kt // 4]
                    nc.tensor.matmul(
                        ps_m[:, mc * 8:(mc + 1) * 8],
                        whop_sb[:, kt * D + mc * 128:kt * D + (mc + 1) * 128],
                        src[:, (kt % 4) * BL:(kt % 4 + 1) * BL],
                        start=(kt == 0), stop=(kt == 11))
            for mc in range(4):
                nc.scalar.activation(m_cur[:, mc * 8:(mc + 1) * 8],
                                     ps_m[:, mc * 8:(mc + 1) * 8], AF.Relu,
                                     bias=bhop_sb[:, hop * 4 + mc:hop * 4 + mc + 1])

        # ---- output head ----
        ps_o = pp2.tile([1, BL], F32, tag="small", space="PSUM")
        for kt in range(8):
            src = m_cur if kt < 4 else q_bf
            nc.tensor.matmul(ps_o[:], wo_sb[:, kt:kt + 1],
                             src[:, (kt % 4) * BL:(kt % 4 + 1) * BL],
                             start=(kt == 0), stop=(kt == 7))
        o_sb = wp.tile([1, BL], F32, tag="osb")
        nc.scalar.activation(o_sb[:], ps_o[:], AF.Sigmoid, bias=bo_sb[0:1, 0:1])
        nc.sync.dma_start(d_out, o_sb[:])

        pp2.release()
        pp.release()
        wp.release()
        cp.release()
    nc.compile()
    return nc


# gate order [i, g, f, o]; the g block is scaled 2x host-side so one sigmoid
# computes sigma(2g), and tanh(g) = 2*sigma(2g) - 1 on the DVE.
PERM2 = np.concatenate([np.arange(0, 256), np.arange(512, 768),
                        np.arange(256, 512), np.arange(768, 1024)])


def _permscale(w):
    w = w[:, PERM2].copy()
    w[:, 256:512] *= 2.0
    return w


def _prep(tokens, lengths, emb, Wx_f, Wh_f, b_f, Wx_b, Wh_b, b_b,
          W1, b1, W2, b2, Wr, Ur, br, Wc, Uc, bc, q,
          W_hops, b_hops, Wo, bo):
    bf16 = ml_dtypes.bfloat16
    a = lambda x: np.asarray(x, np.float32)
    tobf = lambda x: a(x).astype(bf16)

    wx = np.stack([_permscale(np.concatenate([a(Wx_f), a(b_f)[None, :]], 0)),
                   _permscale(np.concatenate([a(Wx_b), a(b_b)[None, :]], 0))])
    wh = np.stack([_permscale(a(Wh_f)), _permscale(a(Wh_b))])
    W1f = a(W1)
    W1h0 = np.concatenate([W1f[0:512] + W1f[512:1024],
                           W1f[1024:1536] + W1f[1536:2048]], 0)
    w1full = np.concatenate([W1f, W1h0], 0)  # [3072, 300]
    wrc = np.stack([np.concatenate([a(Wr), a(br)[None, :]], 0),
                    np.concatenate([a(Wc), a(bc)[None, :]], 0)])
    uu = np.stack([a(Ur), a(Uc)])
    b1T = np.zeros((128, 3), np.float32)
    w2c = np.zeros((128, 3), np.float32)
    for k in range(3):
        n = EK[k]
        b1T[:n, k] = a(b1)[k * 128:k * 128 + n]
        w2c[:n, k] = a(W2)[k * 128:k * 128 + n, 0]
    bhopT = np.zeros((128, NH * 4), np.float32)
    for i in range(NH):
        for mc in range(4):
            bhopT[:, i * 4 + mc] = a(b_hops)[i, mc * 128:(mc + 1) * 128]
    woc = a(Wo)[:, 0].reshape(8, 128).T.copy()
    shared = dict(
        emb=a(emb), wx=tobf(wx), wh=tobf(wh), w1=tobf(w1full), b1T=b1T, w2=tobf(w2c),
        wrc=tobf(wrc), uu=tobf(uu), whops=tobf(W_hops), bhopT=bhopT, wo=tobf(woc),
        bo=a(bo).reshape(1, 1),
        sel=np.kron(np.eye(BL, dtype=np.float32), np.ones((1, 128), np.float32)
                    ).astype(bf16),
    )
    tokens, lengths, q = np.asarray(tokens), np.asarray(lengths), a(q)
    in_maps = []
    for c in range(NC):
        sl = slice(c * BL, (c + 1) * BL)
        in_maps.append(dict(
            shared,
            tokT=tokens[sl].T.astype(np.int32).copy(),
            negmask=np.where(np.arange(T)[None, :] < lengths[sl][:, None],
                             0.0, -1e9).astype(np.float32).T.copy(),
            qT=q[sl].T.reshape(4, 128, BL).transpose(1, 0, 2).reshape(128, 4 * BL).copy(),
        ))
    return in_maps


def kernel(_trace=False, **inputs):
    if "nc" not in _CACHE:
        _CACHE["nc"] = _build()
    nc = _CACHE["nc"]
    in_maps = _prep(**inputs)
    res = bass_utils.run_bass_kernel_spmd(nc, in_maps, core_ids=list(range(NC)),
                                          trace=_trace)
    out = np.concatenate([np.asarray(res.results[c]["out"]).reshape(BL)
                          for c in range(NC)])
    if _trace:
        kernel.last_exec_ns = res.exec_time_ns
    return out.astype(np.float32)
